# revision 19
# baseline (speedup 1.0000x reference)
"""Trainium2 Bass kernel for BasicMGU (nn_BasicMGU_53386443489965).

Math (per reference):
    xz = x @ W_k ; xh = x @ W_u
    f_t = sigmoid(xz_t + h @ W_r + b_r)
    c_t = tanh(xh_t + (h*f_t) @ W_ur + b_ur)
    h   = (1-f_t)*h + f_t*c_t        -> return final h  [B, U]

Sharding: data-parallel over batch across 8 cores (B=64 -> 8 per core),
weights replicated.

Per-core design:
  Phase 1 (projections): two GEMMs in fp32r (full PE rate at N=512),
  producing xzT/xhT in DRAM pre-swizzled into the exact per-chunk SBUF
  layout the recurrence consumes (contiguous 512B+ runs per DMA
  descriptor), biases folded in.
  Phase 2 (recurrence): state kept transposed hT [U(part), B(free)].
  Both per-step matmuls run weight-stationary (lhsT = 128x128 weight
  tile in bf16 -> fast weight load, rhs = state in bf16, N=B=8), so no
  per-step transposes are needed and PSUM outputs stay transposed.
  Accumulation groups are kept consecutive per PSUM slice (m-outer,
  k-inner) - interleaving groups gives wrong results on HW.
  Elementwise/activations run on [128, ...] tiles (128 partitions).
"""

import os
import sys
import types

sys.path.insert(0, "/opt/trn_rl_repo")

import numpy as np
import ml_dtypes

import concourse.bass as bass
import concourse.mybir as mybir
import concourse.tile as tile
from concourse import bacc
from concourse.bass_utils import run_bass_kernel_spmd

B, T, D, U = 64, 1024, 512, 512
NCORES = 8
BL = B // NCORES          # batch per core
S = int(os.environ.get("MGU_S", 32))  # recurrence steps per hw-loop iteration
KC = D // 128             # contraction chunks
MC = U // 128             # output-unit chunks
PCOLS = 512               # projection (t,b) columns per block
NBLK = T * BL // PCOLS
NW = S * BL               # free width of one swizzled chunk slab

F32 = mybir.dt.float32
F32R = mybir.dt.float32r
BF16 = mybir.dt.bfloat16

LAST_EXEC_NS = None


def _install_trace_shim():
    """Make `antenv.axon_hooks` importable so trace=True degrades gracefully
    (and, where the axon .so is present, actually captures NTFF profiles)."""
    if "antenv.axon_hooks" in sys.modules:
        return
    mod = types.ModuleType("antenv.axon_hooks")
    holder = [None]
    mod.set_axon_ntff_profile_hook = lambda h: holder.__setitem__(0, h)
    mod.get_axon_ntff_profile_hook = lambda: holder[0]
    sys.modules["antenv.axon_hooks"] = mod
    try:
        if "/root/.axon_site" not in sys.path:
            sys.path.append("/root/.axon_site")
        from trn_agent_boot.trn_boot import _ntff_profile_via_ctypes

        hook = _ntff_profile_via_ctypes("/opt/axon/libaxon_pjrt.so")
        if hook is not None:
            mod.set_axon_ntff_profile_hook(hook)
    except Exception:
        pass


if os.environ.get("MGU_LDWOPT"):
    import concourse.bass_utils as _bu

    _orig_run_command = _bu.run_command

    def _run_command_ldwopt(argv, **kw):
        argv = [
            a.replace("--enable-ldw-opt=false", "--enable-ldw-opt=true")
            for a in argv
        ]
        return _orig_run_command(argv, **kw)

    _bu.run_command = _run_command_ldwopt


def _build():
    nc = bacc.Bacc("TRN2")

    t_total = int(os.environ.get("MGU_TSTEPS", T))
    nch = t_total // S

    xT = nc.dram_tensor("xT", [D, T * BL], F32, kind="ExternalInput")
    Wk = nc.dram_tensor("Wk", [D, U], F32, kind="ExternalInput")
    Wu = nc.dram_tensor("Wu", [D, U], F32, kind="ExternalInput")
    Wr = nc.dram_tensor("Wr", [U, U], BF16, kind="ExternalInput")
    Wur = nc.dram_tensor("Wur", [U, U], BF16, kind="ExternalInput")
    br = nc.dram_tensor("br", [U], F32, kind="ExternalInput")
    bur = nc.dram_tensor("bur", [U], F32, kind="ExternalInput")
    hT_out = nc.dram_tensor("hT_out", [128, MC, BL], F32, kind="ExternalOutput")
    # Swizzled step-input slabs: [chunk, m, partition(u%128), (s b)]
    xzT_d = nc.dram_tensor("xzT_d", [T // S, MC, 128, NW], F32)
    xhT_d = nc.dram_tensor("xhT_d", [T // S, MC, 128, NW], F32)

    ID = mybir.ActivationFunctionType.Identity
    SIG = mybir.ActivationFunctionType.Sigmoid
    TANH = mybir.ActivationFunctionType.Tanh

    with tile.TileContext(nc) as tc:
        with tc.tile_pool(name="consts", bufs=1) as consts:
            # fp32r matmul inputs must be produced by a compute op (walrus
            # verifier rejects DMA-written fp32r operands), so stage via f32
            # tiles and round with a DVE copy.
            Wk_st = consts.tile([128, KC, U], F32)
            nc.sync.dma_start(Wk_st, Wk[:, :].rearrange("(c p) u -> p c u", p=128))
            Wk_sb = consts.tile([128, KC, U], F32R)
            nc.vector.tensor_copy(Wk_sb, Wk_st)
            Wu_st = consts.tile([128, KC, U], F32)
            nc.sync.dma_start(Wu_st, Wu[:, :].rearrange("(c p) u -> p c u", p=128))
            Wu_sb = consts.tile([128, KC, U], F32R)
            nc.vector.tensor_copy(Wu_sb, Wu_st)
            Wr_sb = consts.tile([128, MC, U], BF16)
            nc.sync.dma_start(Wr_sb, Wr[:, :].rearrange("(c p) u -> p c u", p=128))
            Wur_sb = consts.tile([128, MC, U], BF16)
            nc.sync.dma_start(Wur_sb, Wur[:, :].rearrange("(c p) u -> p c u", p=128))
            br_sb = consts.tile([128, MC], F32)
            nc.sync.dma_start(br_sb, br[:].rearrange("(c p) -> p c", p=128))
            bur_sb = consts.tile([128, MC], F32)
            nc.sync.dma_start(bur_sb, bur[:].rearrange("(c p) -> p c", p=128))

            # Two independent batch half-groups, interleaved per step so one
            # group's matmuls fill the PE while the other is in its
            # sigmoid/tanh/blend latency chain.
            BLG = BL // 2
            hTf = [None, None]
            hTb = [None, None]
            for g in range(2):
                hTf[g] = consts.tile([128, MC, BLG], F32, name=f"hTf{g}")
                nc.vector.memset(hTf[g], 0.0)
                hTb[g] = consts.tile([128, MC, BLG], BF16, name=f"hTb{g}")
                nc.vector.memset(hTb[g], 0.0)

            # ---------------- Phase 1: projections ----------------
            with (
                tc.tile_pool(name="proj_in", bufs=2) as pin,
                tc.tile_pool(name="proj_ps", bufs=4, space="PSUM") as pps,
                tc.tile_pool(name="proj_out", bufs=4) as pout,
            ):
                tblk = PCOLS // BL  # timesteps per column block
                assert tblk % S == 0 or S % tblk == 0
                cpb = max(1, tblk // S)  # swizzle chunks per column block
                for j in range(NBLK):
                    xT_st = pin.tile([128, KC, PCOLS], F32, tag="xT_st")
                    nc.sync.dma_start(
                        xT_st,
                        xT[:, j * PCOLS : (j + 1) * PCOLS].rearrange(
                            "(c p) n -> p c n", p=128
                        ),
                    )
                    xT_sb = pin.tile([128, KC, PCOLS], F32R, tag="xT_r")
                    nc.vector.tensor_copy(xT_sb, xT_st)
                    for W_sb, bias_sb, dst in (
                        (Wk_sb, br_sb, xzT_d),
                        (Wu_sb, bur_sb, xhT_d),
                    ):
                        for m in range(MC):
                            ps = pps.tile([128, PCOLS], F32)
                            for k in range(KC):
                                nc.tensor.matmul(
                                    ps,
                                    W_sb[:, k, m * 128 : (m + 1) * 128],
                                    xT_sb[:, k, :],
                                    start=(k == 0),
                                    stop=(k == KC - 1),
                                )
                            o = pout.tile([128, PCOLS], F32)
                            nc.scalar.activation(o, ps, ID, bias=bias_sb[:, m : m + 1])
                            if cpb >= 1 and tblk >= S:
                                nc.sync.dma_start(
                                    dst[j * cpb : (j + 1) * cpb, m, :, :].rearrange(
                                        "tc p n -> p tc n"
                                    ),
                                    o.rearrange("p (tc n) -> p tc n", tc=cpb),
                                )
                            else:  # S > tblk: one block fills part of a chunk
                                nc.sync.dma_start(
                                    dst[
                                        (j * tblk) // S,
                                        m,
                                        :,
                                        (j % (S // tblk)) * PCOLS : (j % (S // tblk))
                                        * PCOLS
                                        + PCOLS,
                                    ],
                                    o,
                                )

            # ---------------- Phase 2: recurrence ----------------
            dbg2 = os.environ.get("MGU_DEBUG2")
            if dbg2:
                f_dbg = nc.dram_tensor(
                    "f_dbg", [t_total, 128, MC, BL], F32, kind="ExternalOutput"
                )
                c_dbg = nc.dram_tensor(
                    "c_dbg", [t_total, 128, MC, BL], F32, kind="ExternalOutput"
                )
                h_dbg = nc.dram_tensor(
                    "h_dbg", [t_total, 128, MC, BL], F32, kind="ExternalOutput"
                )
            with (
                tc.tile_pool(name="rec_in", bufs=2) as rin,
                tc.tile_pool(name="rec_ps1", bufs=2, space="PSUM") as rps1,
                tc.tile_pool(name="rec_ps2", bufs=2, space="PSUM") as rps2,
                tc.tile_pool(name="rec_tmp", bufs=3) as rtmp,
            ):
                with tc.For_i(0, nch, 1, staggered_reset=True) as it:
                    xz_sb = rin.tile([128, 1, MC, NW], F32, tag="xz")
                    nc.sync.dma_start(
                        xz_sb,
                        xzT_d[bass.ds(it, 1), :, :, :].rearrange("o c p n -> p o c n"),
                    )
                    xh_sb = rin.tile([128, 1, MC, NW], F32, tag="xh")
                    nc.sync.dma_start(
                        xh_sb,
                        xhT_d[bass.ds(it, 1), :, :, :].rearrange("o c p n -> p o c n"),
                    )
                    for s in range(S):
                        gsl = [
                            slice(s * BL + g * BLG, s * BL + (g + 1) * BLG)
                            for g in range(2)
                        ]
                        ps1 = [None, None]
                        for g in range(2):
                            # mm1: psum pre-seeded with xz_t, accumulate
                            # h @ W_r on top (m-outer: consecutive
                            # accumulation groups per psum slice).
                            ps1[g] = rps1.tile(
                                [128, MC, BLG], F32, tag=f"ps1_{g}", name=f"ps1{g}"
                            )
                            nc.vector.tensor_copy(ps1[g], xz_sb[:, 0, :, gsl[g]])
                            for m in range(MC):
                                for k in range(KC):
                                    nc.tensor.matmul(
                                        ps1[g][:, m, :],
                                        Wr_sb[:, k, m * 128 : (m + 1) * 128],
                                        hTb[g][:, k, :],
                                        start=False,
                                        stop=(k == KC - 1),
                                    )
                        fT = [None, None]
                        hfb = [None, None]
                        A = [None, None]
                        ps2 = [None, None]
                        for g in range(2):
                            fT[g] = rtmp.tile(
                                [128, MC, BLG], F32, tag=f"fT_{g}", name=f"fT{g}"
                            )
                            nc.scalar.activation(fT[g], ps1[g], SIG)
                            hfb[g] = rtmp.tile(
                                [128, MC, BLG], BF16, tag=f"hfb_{g}", name=f"hfb{g}"
                            )
                            nc.vector.tensor_mul(hfb[g], hTf[g], fT[g])
                            # off critical path: A = h - h*f (exact, fp32)
                            hf32 = rtmp.tile(
                                [128, MC, BLG], F32, tag=f"hf32_{g}", name=f"hf32{g}"
                            )
                            nc.vector.tensor_mul(hf32, hTf[g], fT[g])
                            A[g] = rtmp.tile(
                                [128, MC, BLG], F32, tag=f"A_{g}", name=f"A{g}"
                            )
                            nc.vector.tensor_sub(A[g], hTf[g], hf32)
                            ps2[g] = rps2.tile(
                                [128, MC, BLG], F32, tag=f"ps2_{g}", name=f"ps2{g}"
                            )
                            nc.vector.tensor_copy(ps2[g], xh_sb[:, 0, :, gsl[g]])
                        for g in range(2):
                            for m in range(MC):
                                for k in range(KC):
                                    nc.tensor.matmul(
                                        ps2[g][:, m, :],
                                        Wur_sb[:, k, m * 128 : (m + 1) * 128],
                                        hfb[g][:, k, :],
                                        start=False,
                                        stop=(k == KC - 1),
                                    )
                        for g in range(2):
                            cT = rtmp.tile(
                                [128, MC, BLG], F32, tag=f"cT_{g}", name=f"cT{g}"
                            )
                            nc.scalar.activation(cT, ps2[g], TANH)
                            # chain: e = f*c ; h_bf16 = A + e first (unblocks
                            # next step's mm1), fp32 master shadows it.
                            nc.vector.tensor_mul(cT, cT, fT[g])
                            nc.vector.tensor_add(hTb[g], A[g], cT)
                            nc.vector.tensor_add(hTf[g], A[g], cT)

            for g in range(2):
                nc.sync.dma_start(
                    hT_out[:, :, g * BLG : (g + 1) * BLG], hTf[g]
                )

    nc.compile()
    return nc


_NC_CACHE = None


def kernel(x, W_k, W_r, b_r, W_u, W_ur, b_ur):
    global _NC_CACHE, LAST_EXEC_NS
    _install_trace_shim()
    if _NC_CACHE is None:
        _NC_CACHE = _build()
    nc = _NC_CACHE

    x = np.ascontiguousarray(np.asarray(x, dtype=np.float32))
    Wr_b = np.asarray(W_r, dtype=np.float32).astype(ml_dtypes.bfloat16)
    Wur_b = np.asarray(W_ur, dtype=np.float32).astype(ml_dtypes.bfloat16)
    Wk_f = np.ascontiguousarray(np.asarray(W_k, dtype=np.float32))
    Wu_f = np.ascontiguousarray(np.asarray(W_u, dtype=np.float32))
    br_f = np.ascontiguousarray(np.asarray(b_r, dtype=np.float32))
    bur_f = np.ascontiguousarray(np.asarray(b_ur, dtype=np.float32))

    in_maps = []
    for c in range(NCORES):
        xc = x[c * BL : (c + 1) * BL]  # [BL, T, D]
        xTc = np.ascontiguousarray(xc.transpose(2, 1, 0).reshape(D, T * BL))
        in_maps.append(
            {
                "xT": xTc,
                "Wk": Wk_f,
                "Wu": Wu_f,
                "Wr": Wr_b,
                "Wur": Wur_b,
                "br": br_f,
                "bur": bur_f,
            }
        )

    trace = bool(os.environ.get("BASS_TRACE"))
    res = run_bass_kernel_spmd(
        nc, in_maps, core_ids=list(range(NCORES)), trace=trace
    )
    LAST_EXEC_NS = res.exec_time_ns

    out = np.empty((B, U), dtype=np.float32)
    for c in range(NCORES):
        hT = res.results[c]["hT_out"]  # [128, MC, BL]
        out[c * BL : (c + 1) * BL] = hT.transpose(2, 1, 0).reshape(BL, U)
    return out


# revision 21
# speedup vs baseline: 1.1310x; 1.1310x over previous
"""Trainium2 Bass kernel for BasicMGU (nn_BasicMGU_53386443489965).

Math (per reference):
    xz = x @ W_k ; xh = x @ W_u
    f_t = sigmoid(xz_t + h @ W_r + b_r)
    c_t = tanh(xh_t + (h*f_t) @ W_ur + b_ur)
    h   = (1-f_t)*h + f_t*c_t        -> return final h  [B, U]

Sharding: data-parallel over batch across 8 cores (B=64 -> 8 per core),
weights replicated.

Per-core design:
  Phase 1 (projections): two GEMMs in fp32r (full PE rate at N=512),
  producing xzT/xhT in DRAM pre-swizzled into the exact per-chunk SBUF
  layout the recurrence consumes (contiguous 512B+ runs per DMA
  descriptor), biases folded in.
  Phase 2 (recurrence): state kept transposed hT [U(part), B(free)].
  Both per-step matmuls run weight-stationary (lhsT = 128x128 weight
  tile in bf16 -> fast weight load, rhs = state in bf16, N=B=8), so no
  per-step transposes are needed and PSUM outputs stay transposed.
  Accumulation groups are kept consecutive per PSUM slice (m-outer,
  k-inner) - interleaving groups gives wrong results on HW.
  Elementwise/activations run on [128, ...] tiles (128 partitions).
"""

import os
import sys
import types

sys.path.insert(0, "/opt/trn_rl_repo")

import numpy as np
import ml_dtypes

import concourse.bass as bass
import concourse.mybir as mybir
import concourse.tile as tile
from concourse import bacc
from concourse.bass_utils import run_bass_kernel_spmd

B, T, D, U = 64, 1024, 512, 512
NCORES = 8
BL = B // NCORES          # batch per core
S = int(os.environ.get("MGU_S", 32))  # recurrence steps per hw-loop iteration
KC = D // 128             # contraction chunks
MC = U // 128             # output-unit chunks
PCOLS = 512               # projection (t,b) columns per block
NBLK = T * BL // PCOLS
NW = S * BL               # free width of one swizzled chunk slab

F32 = mybir.dt.float32
F32R = mybir.dt.float32r
BF16 = mybir.dt.bfloat16

LAST_EXEC_NS = None


def _install_trace_shim():
    """Make `antenv.axon_hooks` importable so trace=True degrades gracefully
    (and, where the axon .so is present, actually captures NTFF profiles)."""
    if "antenv.axon_hooks" in sys.modules:
        return
    mod = types.ModuleType("antenv.axon_hooks")
    holder = [None]
    mod.set_axon_ntff_profile_hook = lambda h: holder.__setitem__(0, h)
    mod.get_axon_ntff_profile_hook = lambda: holder[0]
    sys.modules["antenv.axon_hooks"] = mod
    try:
        if "/root/.axon_site" not in sys.path:
            sys.path.append("/root/.axon_site")
        from trn_agent_boot.trn_boot import _ntff_profile_via_ctypes

        hook = _ntff_profile_via_ctypes("/opt/axon/libaxon_pjrt.so")
        if hook is not None:
            mod.set_axon_ntff_profile_hook(hook)
    except Exception:
        pass


if os.environ.get("MGU_LDWOPT"):
    import concourse.bass_utils as _bu

    _orig_run_command = _bu.run_command

    def _run_command_ldwopt(argv, **kw):
        argv = [
            a.replace("--enable-ldw-opt=false", "--enable-ldw-opt=true")
            for a in argv
        ]
        return _orig_run_command(argv, **kw)

    _bu.run_command = _run_command_ldwopt


def _build():
    nc = bacc.Bacc("TRN2")

    t_total = int(os.environ.get("MGU_TSTEPS", T))
    nch = t_total // S

    xT = nc.dram_tensor("xT", [D, T * BL], F32, kind="ExternalInput")
    Wk = nc.dram_tensor("Wk", [D, U], F32, kind="ExternalInput")
    Wu = nc.dram_tensor("Wu", [D, U], F32, kind="ExternalInput")
    Wr = nc.dram_tensor("Wr", [U, U], BF16, kind="ExternalInput")
    Wur = nc.dram_tensor("Wur", [U, U], BF16, kind="ExternalInput")
    br = nc.dram_tensor("br", [U], F32, kind="ExternalInput")
    bur = nc.dram_tensor("bur", [U], F32, kind="ExternalInput")
    hT_out = nc.dram_tensor("hT_out", [128, MC, BL], F32, kind="ExternalOutput")
    # Swizzled step-input slabs: [chunk, m, partition(u%128), (s b)]
    xzT_d = nc.dram_tensor("xzT_d", [T // S, MC, 128, NW], F32)
    xhT_d = nc.dram_tensor("xhT_d", [T // S, MC, 128, NW], F32)

    ID = mybir.ActivationFunctionType.Identity
    SIG = mybir.ActivationFunctionType.Sigmoid
    TANH = mybir.ActivationFunctionType.Tanh

    with tile.TileContext(nc) as tc:
        with tc.tile_pool(name="consts", bufs=1) as consts:
            # fp32r matmul inputs must be produced by a compute op (walrus
            # verifier rejects DMA-written fp32r operands), so stage via f32
            # tiles and round with a DVE copy.
            Wk_st = consts.tile([128, KC, U], F32)
            nc.sync.dma_start(Wk_st, Wk[:, :].rearrange("(c p) u -> p c u", p=128))
            Wk_sb = consts.tile([128, KC, U], F32R)
            nc.vector.tensor_copy(Wk_sb, Wk_st)
            Wu_st = consts.tile([128, KC, U], F32)
            nc.sync.dma_start(Wu_st, Wu[:, :].rearrange("(c p) u -> p c u", p=128))
            Wu_sb = consts.tile([128, KC, U], F32R)
            nc.vector.tensor_copy(Wu_sb, Wu_st)
            Wr_sb = consts.tile([128, MC, U], BF16)
            nc.sync.dma_start(Wr_sb, Wr[:, :].rearrange("(c p) u -> p c u", p=128))
            Wur_sb = consts.tile([128, MC, U], BF16)
            nc.sync.dma_start(Wur_sb, Wur[:, :].rearrange("(c p) u -> p c u", p=128))
            br_sb = consts.tile([128, MC], F32)
            nc.sync.dma_start(br_sb, br[:].rearrange("(c p) -> p c", p=128))
            bur_sb = consts.tile([128, MC], F32)
            nc.sync.dma_start(bur_sb, bur[:].rearrange("(c p) -> p c", p=128))

            hTf = consts.tile([128, MC, BL], F32)
            nc.vector.memset(hTf, 0.0)
            hTb = consts.tile([128, MC, BL], BF16)
            nc.vector.memset(hTb, 0.0)

            # ---------------- Phase 1: projections ----------------
            with (
                tc.tile_pool(name="proj_in", bufs=2) as pin,
                tc.tile_pool(name="proj_ps", bufs=4, space="PSUM") as pps,
                tc.tile_pool(name="proj_out", bufs=4) as pout,
            ):
                tblk = PCOLS // BL  # timesteps per column block
                assert tblk % S == 0 or S % tblk == 0
                cpb = max(1, tblk // S)  # swizzle chunks per column block
                for j in range(NBLK):
                    xT_st = pin.tile([128, KC, PCOLS], F32, tag="xT_st")
                    nc.sync.dma_start(
                        xT_st,
                        xT[:, j * PCOLS : (j + 1) * PCOLS].rearrange(
                            "(c p) n -> p c n", p=128
                        ),
                    )
                    xT_sb = pin.tile([128, KC, PCOLS], F32R, tag="xT_r")
                    nc.vector.tensor_copy(xT_sb, xT_st)
                    for W_sb, bias_sb, dst in (
                        (Wk_sb, br_sb, xzT_d),
                        (Wu_sb, bur_sb, xhT_d),
                    ):
                        for m in range(MC):
                            ps = pps.tile([128, PCOLS], F32)
                            for k in range(KC):
                                nc.tensor.matmul(
                                    ps,
                                    W_sb[:, k, m * 128 : (m + 1) * 128],
                                    xT_sb[:, k, :],
                                    start=(k == 0),
                                    stop=(k == KC - 1),
                                )
                            o = pout.tile([128, PCOLS], F32)
                            nc.scalar.activation(o, ps, ID, bias=bias_sb[:, m : m + 1])
                            if cpb >= 1 and tblk >= S:
                                nc.sync.dma_start(
                                    dst[j * cpb : (j + 1) * cpb, m, :, :].rearrange(
                                        "tc p n -> p tc n"
                                    ),
                                    o.rearrange("p (tc n) -> p tc n", tc=cpb),
                                )
                            else:  # S > tblk: one block fills part of a chunk
                                nc.sync.dma_start(
                                    dst[
                                        (j * tblk) // S,
                                        m,
                                        :,
                                        (j % (S // tblk)) * PCOLS : (j % (S // tblk))
                                        * PCOLS
                                        + PCOLS,
                                    ],
                                    o,
                                )

            # ---------------- Phase 2: recurrence ----------------
            dbg2 = os.environ.get("MGU_DEBUG2")
            if dbg2:
                f_dbg = nc.dram_tensor(
                    "f_dbg", [t_total, 128, MC, BL], F32, kind="ExternalOutput"
                )
                c_dbg = nc.dram_tensor(
                    "c_dbg", [t_total, 128, MC, BL], F32, kind="ExternalOutput"
                )
                h_dbg = nc.dram_tensor(
                    "h_dbg", [t_total, 128, MC, BL], F32, kind="ExternalOutput"
                )
            with (
                tc.tile_pool(name="rec_in", bufs=2) as rin,
                tc.tile_pool(name="rec_ps1", bufs=2, space="PSUM") as rps1,
                tc.tile_pool(name="rec_ps2", bufs=2, space="PSUM") as rps2,
                tc.tile_pool(name="rec_tmp", bufs=3) as rtmp,
            ):
                with tc.For_i(0, nch, 1, staggered_reset=True) as it:
                    xz_sb = rin.tile([128, 1, MC, NW], F32, tag="xz")
                    nc.sync.dma_start(
                        xz_sb,
                        xzT_d[bass.ds(it, 1), :, :, :].rearrange("o c p n -> p o c n"),
                    )
                    xh_sb = rin.tile([128, 1, MC, NW], F32, tag="xh")
                    nc.sync.dma_start(
                        xh_sb,
                        xhT_d[bass.ds(it, 1), :, :, :].rearrange("o c p n -> p o c n"),
                    )
                    # chunk head: step 0's mm1 runs from the bf16 state
                    # snapshot saved at the previous chunk boundary.
                    ps1 = rps1.tile([128, MC, BL], F32, tag="ps1", name="ps1h")
                    nc.vector.tensor_copy(ps1, xz_sb[:, 0, :, 0:BL])
                    for m in range(MC):
                        for k in range(KC):
                            nc.tensor.matmul(
                                ps1[:, m, :],
                                Wr_sb[:, k, m * 128 : (m + 1) * 128],
                                hTb[:, k, :],
                                start=False,
                                stop=(k == KC - 1),
                            )
                    for s in range(S):
                        # chain: sigmoid -> hf (bf16) -> mm2 -> tanh -> e
                        # -> next step's mm1b. The state update h' = A + e
                        # and next mm1's A-part run off the chain:
                        # z1(t+1) = xz(t+1) + A@W_r + e@W_r  (linearity).
                        fT = rtmp.tile([128, MC, BL], F32, tag="fT")
                        nc.scalar.activation(fT, ps1, SIG)
                        hfb = rtmp.tile([128, MC, BL], BF16, tag="hfb")
                        nc.vector.tensor_mul(hfb, hTf, fT)
                        Ab = rtmp.tile([128, MC, BL], BF16, tag="Ab")
                        nc.vector.tensor_sub(Ab, hTf, hfb)
                        ps2 = rps2.tile([128, MC, BL], F32, tag="ps2")
                        nc.vector.tensor_copy(ps2, xh_sb[:, 0, :, s * BL : (s + 1) * BL])
                        for m in range(MC):
                            for k in range(KC):
                                nc.tensor.matmul(
                                    ps2[:, m, :],
                                    Wur_sb[:, k, m * 128 : (m + 1) * 128],
                                    hfb[:, k, :],
                                    start=False,
                                    stop=(k == KC - 1),
                                )
                        ps1n = None
                        if s < S - 1:
                            nsl = slice((s + 1) * BL, (s + 2) * BL)
                            ps1n = rps1.tile([128, MC, BL], F32, tag="ps1", name="ps1n")
                            nc.vector.tensor_copy(ps1n, xz_sb[:, 0, :, nsl])
                            for m in range(MC):
                                for k in range(KC):
                                    nc.tensor.matmul(
                                        ps1n[:, m, :],
                                        Wr_sb[:, k, m * 128 : (m + 1) * 128],
                                        Ab[:, k, :],
                                        start=False,
                                        stop=False,
                                    )
                        cT = rtmp.tile([128, MC, BL], F32, tag="cT")
                        nc.scalar.activation(cT, ps2, TANH)
                        eb = rtmp.tile([128, MC, BL], BF16, tag="eb")
                        nc.vector.tensor_mul(eb, cT, fT)
                        if s < S - 1:
                            for m in range(MC):
                                for k in range(KC):
                                    nc.tensor.matmul(
                                        ps1n[:, m, :],
                                        Wr_sb[:, k, m * 128 : (m + 1) * 128],
                                        eb[:, k, :],
                                        start=False,
                                        stop=(k == KC - 1),
                                    )
                        nc.vector.tensor_add(hTf, Ab, eb)
                        if s == S - 1:
                            nc.vector.tensor_add(hTb, Ab, eb)
                        ps1 = ps1n

            nc.sync.dma_start(hT_out[:, :, :], hTf)

    nc.compile()
    return nc


_NC_CACHE = None


def kernel(x, W_k, W_r, b_r, W_u, W_ur, b_ur):
    global _NC_CACHE, LAST_EXEC_NS
    _install_trace_shim()
    if _NC_CACHE is None:
        _NC_CACHE = _build()
    nc = _NC_CACHE

    x = np.ascontiguousarray(np.asarray(x, dtype=np.float32))
    Wr_b = np.asarray(W_r, dtype=np.float32).astype(ml_dtypes.bfloat16)
    Wur_b = np.asarray(W_ur, dtype=np.float32).astype(ml_dtypes.bfloat16)
    Wk_f = np.ascontiguousarray(np.asarray(W_k, dtype=np.float32))
    Wu_f = np.ascontiguousarray(np.asarray(W_u, dtype=np.float32))
    br_f = np.ascontiguousarray(np.asarray(b_r, dtype=np.float32))
    bur_f = np.ascontiguousarray(np.asarray(b_ur, dtype=np.float32))

    in_maps = []
    for c in range(NCORES):
        xc = x[c * BL : (c + 1) * BL]  # [BL, T, D]
        xTc = np.ascontiguousarray(xc.transpose(2, 1, 0).reshape(D, T * BL))
        in_maps.append(
            {
                "xT": xTc,
                "Wk": Wk_f,
                "Wu": Wu_f,
                "Wr": Wr_b,
                "Wur": Wur_b,
                "br": br_f,
                "bur": bur_f,
            }
        )

    trace = bool(os.environ.get("BASS_TRACE"))
    res = run_bass_kernel_spmd(
        nc, in_maps, core_ids=list(range(NCORES)), trace=trace
    )
    LAST_EXEC_NS = res.exec_time_ns

    out = np.empty((B, U), dtype=np.float32)
    for c in range(NCORES):
        hT = res.results[c]["hT_out"]  # [128, MC, BL]
        out[c * BL : (c + 1) * BL] = hT.transpose(2, 1, 0).reshape(BL, U)
    return out


# revision 22
# speedup vs baseline: 1.1705x; 1.0349x over previous
"""Trainium2 Bass kernel for BasicMGU (nn_BasicMGU_53386443489965).

Math (per reference):
    xz = x @ W_k ; xh = x @ W_u
    f_t = sigmoid(xz_t + h @ W_r + b_r)
    c_t = tanh(xh_t + (h*f_t) @ W_ur + b_ur)
    h   = (1-f_t)*h + f_t*c_t        -> return final h  [B, U]

Sharding: data-parallel over batch across 8 cores (B=64 -> 8 per core),
weights replicated.

Per-core design:
  Phase 1 (projections): two GEMMs in fp32r (full PE rate at N=512),
  producing xzT/xhT in DRAM pre-swizzled into the exact per-chunk SBUF
  layout the recurrence consumes (contiguous 512B+ runs per DMA
  descriptor), biases folded in.
  Phase 2 (recurrence): state kept transposed hT [U(part), B(free)].
  Both per-step matmuls run weight-stationary (lhsT = 128x128 weight
  tile in bf16 -> fast weight load, rhs = state in bf16, N=B=8), so no
  per-step transposes are needed and PSUM outputs stay transposed.
  Accumulation groups are kept consecutive per PSUM slice (m-outer,
  k-inner) - interleaving groups gives wrong results on HW.
  Elementwise/activations run on [128, ...] tiles (128 partitions).
"""

import os
import sys
import types

sys.path.insert(0, "/opt/trn_rl_repo")

import numpy as np
import ml_dtypes

import concourse.bass as bass
import concourse.mybir as mybir
import concourse.tile as tile
from concourse import bacc
from concourse.bass_utils import run_bass_kernel_spmd

B, T, D, U = 64, 1024, 512, 512
NCORES = 8
BL = B // NCORES          # batch per core
S = int(os.environ.get("MGU_S", 64))  # recurrence steps per hw-loop iteration
KC = D // 128             # contraction chunks
MC = U // 128             # output-unit chunks
PCOLS = 512               # projection (t,b) columns per block
NBLK = T * BL // PCOLS
NW = S * BL               # free width of one swizzled chunk slab

F32 = mybir.dt.float32
F32R = mybir.dt.float32r
BF16 = mybir.dt.bfloat16

LAST_EXEC_NS = None


def _install_trace_shim():
    """Make `antenv.axon_hooks` importable so trace=True degrades gracefully
    (and, where the axon .so is present, actually captures NTFF profiles)."""
    if "antenv.axon_hooks" in sys.modules:
        return
    mod = types.ModuleType("antenv.axon_hooks")
    holder = [None]
    mod.set_axon_ntff_profile_hook = lambda h: holder.__setitem__(0, h)
    mod.get_axon_ntff_profile_hook = lambda: holder[0]
    sys.modules["antenv.axon_hooks"] = mod
    try:
        if "/root/.axon_site" not in sys.path:
            sys.path.append("/root/.axon_site")
        from trn_agent_boot.trn_boot import _ntff_profile_via_ctypes

        hook = _ntff_profile_via_ctypes("/opt/axon/libaxon_pjrt.so")
        if hook is not None:
            mod.set_axon_ntff_profile_hook(hook)
    except Exception:
        pass


if os.environ.get("MGU_LDWOPT"):
    import concourse.bass_utils as _bu

    _orig_run_command = _bu.run_command

    def _run_command_ldwopt(argv, **kw):
        argv = [
            a.replace("--enable-ldw-opt=false", "--enable-ldw-opt=true")
            for a in argv
        ]
        return _orig_run_command(argv, **kw)

    _bu.run_command = _run_command_ldwopt


def _build():
    nc = bacc.Bacc("TRN2")

    t_total = int(os.environ.get("MGU_TSTEPS", T))
    nch = t_total // S

    xT = nc.dram_tensor("xT", [D, T * BL], F32, kind="ExternalInput")
    Wk = nc.dram_tensor("Wk", [D, U], F32, kind="ExternalInput")
    Wu = nc.dram_tensor("Wu", [D, U], F32, kind="ExternalInput")
    Wr = nc.dram_tensor("Wr", [U, U], BF16, kind="ExternalInput")
    Wur = nc.dram_tensor("Wur", [U, U], BF16, kind="ExternalInput")
    br = nc.dram_tensor("br", [U], F32, kind="ExternalInput")
    bur = nc.dram_tensor("bur", [U], F32, kind="ExternalInput")
    hT_out = nc.dram_tensor("hT_out", [128, MC, BL], F32, kind="ExternalOutput")
    # Swizzled step-input slabs: [chunk, m, partition(u%128), (s b)]
    xzT_d = nc.dram_tensor("xzT_d", [T // S, MC, 128, NW], F32)
    xhT_d = nc.dram_tensor("xhT_d", [T // S, MC, 128, NW], F32)

    ID = mybir.ActivationFunctionType.Identity
    SIG = mybir.ActivationFunctionType.Sigmoid
    TANH = mybir.ActivationFunctionType.Tanh

    with tile.TileContext(nc) as tc:
        with tc.tile_pool(name="consts", bufs=1) as consts:
            # fp32r matmul inputs must be produced by a compute op (walrus
            # verifier rejects DMA-written fp32r operands), so stage via f32
            # tiles and round with a DVE copy.
            Wk_st = consts.tile([128, KC, U], F32)
            nc.sync.dma_start(Wk_st, Wk[:, :].rearrange("(c p) u -> p c u", p=128))
            Wk_sb = consts.tile([128, KC, U], F32R)
            nc.vector.tensor_copy(Wk_sb, Wk_st)
            Wu_st = consts.tile([128, KC, U], F32)
            nc.sync.dma_start(Wu_st, Wu[:, :].rearrange("(c p) u -> p c u", p=128))
            Wu_sb = consts.tile([128, KC, U], F32R)
            nc.vector.tensor_copy(Wu_sb, Wu_st)
            Wr_sb = consts.tile([128, MC, U], BF16)
            nc.sync.dma_start(Wr_sb, Wr[:, :].rearrange("(c p) u -> p c u", p=128))
            Wur_sb = consts.tile([128, MC, U], BF16)
            nc.sync.dma_start(Wur_sb, Wur[:, :].rearrange("(c p) u -> p c u", p=128))
            br_sb = consts.tile([128, MC], F32)
            nc.sync.dma_start(br_sb, br[:].rearrange("(c p) -> p c", p=128))
            bur_sb = consts.tile([128, MC], F32)
            nc.sync.dma_start(bur_sb, bur[:].rearrange("(c p) -> p c", p=128))

            hTf = consts.tile([128, MC, BL], F32)
            nc.vector.memset(hTf, 0.0)
            hTb = consts.tile([128, MC, BL], BF16)
            nc.vector.memset(hTb, 0.0)

            # ---------------- Phase 1: projections ----------------
            with (
                tc.tile_pool(name="proj_in", bufs=2) as pin,
                tc.tile_pool(name="proj_ps", bufs=6, space="PSUM") as pps,
                tc.tile_pool(name="proj_out", bufs=6) as pout,
            ):
                tblk = PCOLS // BL  # timesteps per column block
                assert tblk % S == 0 or S % tblk == 0
                cpb = max(1, tblk // S)  # swizzle chunks per column block
                for j in range(NBLK):
                    xT_st = pin.tile([128, KC, PCOLS], F32, tag="xT_st")
                    nc.sync.dma_start(
                        xT_st,
                        xT[:, j * PCOLS : (j + 1) * PCOLS].rearrange(
                            "(c p) n -> p c n", p=128
                        ),
                    )
                    xT_sb = pin.tile([128, KC, PCOLS], F32R, tag="xT_r")
                    nc.vector.tensor_copy(xT_sb, xT_st)
                    for W_sb, bias_sb, dst in (
                        (Wk_sb, br_sb, xzT_d),
                        (Wu_sb, bur_sb, xhT_d),
                    ):
                        for m in range(MC):
                            ps = pps.tile([128, PCOLS], F32)
                            for k in range(KC):
                                nc.tensor.matmul(
                                    ps,
                                    W_sb[:, k, m * 128 : (m + 1) * 128],
                                    xT_sb[:, k, :],
                                    start=(k == 0),
                                    stop=(k == KC - 1),
                                )
                            o = pout.tile([128, PCOLS], F32)
                            nc.scalar.activation(o, ps, ID, bias=bias_sb[:, m : m + 1])
                            if cpb >= 1 and tblk >= S:
                                nc.sync.dma_start(
                                    dst[j * cpb : (j + 1) * cpb, m, :, :].rearrange(
                                        "tc p n -> p tc n"
                                    ),
                                    o.rearrange("p (tc n) -> p tc n", tc=cpb),
                                )
                            else:  # S > tblk: one block fills part of a chunk
                                nc.sync.dma_start(
                                    dst[
                                        (j * tblk) // S,
                                        m,
                                        :,
                                        (j % (S // tblk)) * PCOLS : (j % (S // tblk))
                                        * PCOLS
                                        + PCOLS,
                                    ],
                                    o,
                                )

            # ---------------- Phase 2: recurrence ----------------
            dbg2 = os.environ.get("MGU_DEBUG2")
            if dbg2:
                f_dbg = nc.dram_tensor(
                    "f_dbg", [t_total, 128, MC, BL], F32, kind="ExternalOutput"
                )
                c_dbg = nc.dram_tensor(
                    "c_dbg", [t_total, 128, MC, BL], F32, kind="ExternalOutput"
                )
                h_dbg = nc.dram_tensor(
                    "h_dbg", [t_total, 128, MC, BL], F32, kind="ExternalOutput"
                )
            with (
                tc.tile_pool(name="rec_in", bufs=2) as rin,
                tc.tile_pool(name="rec_ps1", bufs=2, space="PSUM") as rps1,
                tc.tile_pool(name="rec_ps2", bufs=2, space="PSUM") as rps2,
                tc.tile_pool(name="rec_tmp", bufs=3) as rtmp,
            ):
                with tc.For_i(0, nch, 1, staggered_reset=True) as it:
                    xz_sb = rin.tile([128, 1, MC, NW], F32, tag="xz")
                    nc.sync.dma_start(
                        xz_sb,
                        xzT_d[bass.ds(it, 1), :, :, :].rearrange("o c p n -> p o c n"),
                    )
                    xh_sb = rin.tile([128, 1, MC, NW], F32, tag="xh")
                    nc.sync.dma_start(
                        xh_sb,
                        xhT_d[bass.ds(it, 1), :, :, :].rearrange("o c p n -> p o c n"),
                    )
                    # chunk head: step 0's mm1 runs from the bf16 state
                    # snapshot saved at the previous chunk boundary.
                    ps1 = rps1.tile([128, MC, BL], F32, tag="ps1", name="ps1h")
                    nc.vector.tensor_copy(ps1, xz_sb[:, 0, :, 0:BL])
                    for m in range(MC):
                        for k in range(KC):
                            nc.tensor.matmul(
                                ps1[:, m, :],
                                Wr_sb[:, k, m * 128 : (m + 1) * 128],
                                hTb[:, k, :],
                                start=False,
                                stop=(k == KC - 1),
                            )
                    for s in range(S):
                        # chain: sigmoid -> hf (bf16) -> mm2 -> tanh -> e
                        # -> next step's mm1b. The state update h' = A + e
                        # and next mm1's A-part run off the chain:
                        # z1(t+1) = xz(t+1) + A@W_r + e@W_r  (linearity).
                        fT = rtmp.tile([128, MC, BL], F32, tag="fT")
                        nc.scalar.activation(fT, ps1, SIG)
                        hfb = rtmp.tile([128, MC, BL], BF16, tag="hfb")
                        nc.vector.tensor_mul(hfb, hTf, fT)
                        Ab = rtmp.tile([128, MC, BL], BF16, tag="Ab")
                        nc.vector.tensor_sub(Ab, hTf, hfb)
                        ps2 = rps2.tile([128, MC, BL], F32, tag="ps2")
                        nc.vector.tensor_copy(ps2, xh_sb[:, 0, :, s * BL : (s + 1) * BL])
                        for m in range(MC):
                            for k in range(KC):
                                nc.tensor.matmul(
                                    ps2[:, m, :],
                                    Wur_sb[:, k, m * 128 : (m + 1) * 128],
                                    hfb[:, k, :],
                                    start=False,
                                    stop=(k == KC - 1),
                                )
                        ps1n = None
                        if s < S - 1:
                            nsl = slice((s + 1) * BL, (s + 2) * BL)
                            ps1n = rps1.tile([128, MC, BL], F32, tag="ps1", name="ps1n")
                            nc.vector.tensor_copy(ps1n, xz_sb[:, 0, :, nsl])
                            for m in range(MC):
                                for k in range(KC):
                                    nc.tensor.matmul(
                                        ps1n[:, m, :],
                                        Wr_sb[:, k, m * 128 : (m + 1) * 128],
                                        Ab[:, k, :],
                                        start=False,
                                        stop=False,
                                    )
                        cT = rtmp.tile([128, MC, BL], F32, tag="cT")
                        nc.scalar.activation(cT, ps2, TANH)
                        eb = rtmp.tile([128, MC, BL], BF16, tag="eb")
                        nc.vector.tensor_mul(eb, cT, fT)
                        if s < S - 1:
                            for m in range(MC):
                                for k in range(KC):
                                    nc.tensor.matmul(
                                        ps1n[:, m, :],
                                        Wr_sb[:, k, m * 128 : (m + 1) * 128],
                                        eb[:, k, :],
                                        start=False,
                                        stop=(k == KC - 1),
                                    )
                        nc.vector.tensor_add(hTf, Ab, eb)
                        if s == S - 1:
                            nc.vector.tensor_add(hTb, Ab, eb)
                        ps1 = ps1n

            nc.sync.dma_start(hT_out[:, :, :], hTf)

    nc.compile()
    return nc


_NC_CACHE = None


def kernel(x, W_k, W_r, b_r, W_u, W_ur, b_ur):
    global _NC_CACHE, LAST_EXEC_NS
    _install_trace_shim()
    if _NC_CACHE is None:
        _NC_CACHE = _build()
    nc = _NC_CACHE

    x = np.ascontiguousarray(np.asarray(x, dtype=np.float32))
    Wr_b = np.asarray(W_r, dtype=np.float32).astype(ml_dtypes.bfloat16)
    Wur_b = np.asarray(W_ur, dtype=np.float32).astype(ml_dtypes.bfloat16)
    Wk_f = np.ascontiguousarray(np.asarray(W_k, dtype=np.float32))
    Wu_f = np.ascontiguousarray(np.asarray(W_u, dtype=np.float32))
    br_f = np.ascontiguousarray(np.asarray(b_r, dtype=np.float32))
    bur_f = np.ascontiguousarray(np.asarray(b_ur, dtype=np.float32))

    in_maps = []
    for c in range(NCORES):
        xc = x[c * BL : (c + 1) * BL]  # [BL, T, D]
        xTc = np.ascontiguousarray(xc.transpose(2, 1, 0).reshape(D, T * BL))
        in_maps.append(
            {
                "xT": xTc,
                "Wk": Wk_f,
                "Wu": Wu_f,
                "Wr": Wr_b,
                "Wur": Wur_b,
                "br": br_f,
                "bur": bur_f,
            }
        )

    trace = bool(os.environ.get("BASS_TRACE"))
    res = run_bass_kernel_spmd(
        nc, in_maps, core_ids=list(range(NCORES)), trace=trace
    )
    LAST_EXEC_NS = res.exec_time_ns

    out = np.empty((B, U), dtype=np.float32)
    for c in range(NCORES):
        hT = res.results[c]["hT_out"]  # [128, MC, BL]
        out[c * BL : (c + 1) * BL] = hT.transpose(2, 1, 0).reshape(BL, U)
    return out
